# revision 8
# baseline (speedup 1.0000x reference)
"""Trainium2 Bass kernel for HDSLinear (gumbel top-2-of-4 masked linear).

Strategy (column-parallel, per sharding hint):
  - Shard weight/scores/noise_u/bias along out_features across 8 cores
    (512 rows each); replicate x (uploaded transposed: [d_in, s] layout,
    a pure host-side relayout so the contraction dim lands on SBUF
    partitions without any device-side transpose).
  - Each core computes its mask shard from scores+gumbel noise on device
    (ACT: 2x Ln; DVE: pairwise-compare rank select), applies it to the
    weight shard, transposes the masked weight on-chip (xbar DMA
    transpose, bf16), then runs x @ Wm^T as a PE matmul accumulating
    over d_in, + bias via a K=1 matmul, and streams out [16384, 512].
  - Host concatenates the 8 output shards along out_features.

Matmul dtype is bf16 by default (BASS_MM_MODE=bf16|f32r|fp32).
"""

import os
import sys
import numpy as np
from contextlib import ExitStack

for _p in ("/opt/trn_rl_repo", "/root/.axon_site/_ro/trn_rl_repo"):
    if os.path.isdir(_p) and _p not in sys.path:
        sys.path.insert(0, _p)

import concourse.bass as bass
import concourse.bacc as bacc
import concourse.mybir as mybir
from concourse import tile
from concourse.bass_utils import run_bass_kernel_spmd

F32 = mybir.dt.float32
BF16 = mybir.dt.bfloat16
AF = mybir.ActivationFunctionType
ALU = mybir.AluOpType

B, S, D_IN, D_OUT = 8, 2048, 4096, 4096
N_CORES = 8
S_TOT = B * S                      # 16384
O_SH = D_OUT // N_CORES            # 512 out-features per core
P = 128
EPS = 1e-10

MM_MODE = os.environ.get("BASS_MM_MODE", "bf16")
MM_DT = {"bf16": BF16, "f32r": mybir.dt.float32r, "fp32": F32}[MM_MODE]
MM_IS_F32 = MM_MODE in ("f32r", "fp32")

K_TILES = D_IN // P                # 32 contraction tiles
S_BLK = 512                        # s-columns per phase-2 block
N_BLK = S_TOT // S_BLK             # 32 blocks
KG = 8                             # k-tiles per x staging DMA
O_TILES = O_SH // P                # 4 o-tiles of 128 rows in phase 1
D_HALF = 2048                      # phase-1 d-chunk width

LAST_EXEC_NS = None
_CACHED = {}


def _build_nc():
    nc = bacc.Bacc(None, target_bir_lowering=False)
    xt = nc.declare_dram_parameter("xt", [D_IN, S_TOT], F32, isOutput=False)
    wsh = nc.declare_dram_parameter("wsh", [O_SH, D_IN], F32, isOutput=False)
    ssh = nc.declare_dram_parameter("ssh", [O_SH, D_IN], F32, isOutput=False)
    nsh = nc.declare_dram_parameter("nsh", [O_SH, D_IN], F32, isOutput=False)
    bsh = nc.declare_dram_parameter("bsh", [1, O_SH], F32, isOutput=False)
    out = nc.declare_dram_parameter("out", [S_TOT, O_SH], F32, isOutput=True)

    with tile.TileContext(nc) as tc:
      with tc.tile_pool(name="const", bufs=1) as const:
        # --- persistent tiles ---
        # Masked weight, transposed: wmt[p, k, o] = Wm[o, 128k+p]
        wmt = const.tile([P, K_TILES, O_SH], MM_DT, tag="wmt")
        ones1 = const.tile([1, P], MM_DT, tag="ones1")
        nc.any.memset(ones1[:], 1.0)
        biasT = const.tile([1, O_SH], MM_DT, tag="biasT")
        bias_f32 = const.tile([1, O_SH], F32, tag="bias_f32")
        nc.sync.dma_start(out=bias_f32[:], in_=bsh[:, :])
        nc.vector.tensor_copy(biasT[:], bias_f32[:])
        epsb = const.tile([P, 1], F32, tag="epsb")
        nc.any.memset(epsb[:], EPS)

        # --- phase 1: mask generation + masked weight (transposed) ---
        with (
            tc.tile_pool(name="p1io", bufs=2) as p1io,
            tc.tile_pool(name="p1t", bufs=2) as p1t,
            tc.tile_pool(name="p1c", bufs=2) as p1c,
        ):
            n_half = D_IN // D_HALF
            G_H = D_HALF // 4   # groups per half-chunk
            for ot in range(O_TILES):
                o0 = ot * P
                for h in range(n_half):
                    d0 = h * D_HALF
                    sc = p1io.tile([P, D_HALF], F32, tag="sc")
                    nu = p1io.tile([P, D_HALF], F32, tag="nu")
                    w = p1io.tile([P, D_HALF], F32, tag="w")
                    nc.sync.dma_start(out=sc[:], in_=ssh[o0:o0 + P, d0:d0 + D_HALF])
                    nc.sync.dma_start(out=nu[:], in_=nsh[o0:o0 + P, d0:d0 + D_HALF])
                    nc.sync.dma_start(out=w[:], in_=wsh[o0:o0 + P, d0:d0 + D_HALF])

                    t1 = p1t.tile([P, D_HALF], F32, tag="t1")
                    t2 = p1t.tile([P, D_HALF], F32, tag="t2")
                    y = p1t.tile([P, D_HALF], F32, tag="y")
                    wmb = p1t.tile([P, D_HALF], MM_DT, tag="wmb")
                    # gumbel chain, mirroring jax fp32 op order:
                    # t1 = ln(u + eps); t2 = ln(-t1 + eps); y = scores - t2
                    nc.scalar.activation(t1[:], nu[:], AF.Ln, bias=epsb[:])
                    nc.scalar.activation(t2[:], t1[:], AF.Ln, bias=epsb[:], scale=-1.0)
                    nc.vector.tensor_sub(y[:], sc[:], t2[:])

                    yg = y.rearrange("p (g m) -> p g m", m=4)
                    wg = w.rearrange("p (g m) -> p g m", m=4)
                    wmg = wmb.rearrange("p (g m) -> p g m", m=4)
                    yk = [yg[:, :, k] for k in range(4)]

                    def cmp(a, b):
                        t = p1c.tile([P, G_H], F32, tag=f"ge{a}{b}")
                        nc.vector.tensor_tensor(t[:], yk[a][:], yk[b][:], ALU.is_ge)
                        return t

                    ge01, ge02, ge03 = cmp(0, 1), cmp(0, 2), cmp(0, 3)
                    ge12, ge13, ge23 = cmp(1, 2), cmp(1, 3), cmp(2, 3)

                    def keep_apply(k, terms, thr, op):
                        # sum(terms) (with signs) `op` thr -> *w_k -> wm_k
                        a = p1c.tile([P, G_H], F32, tag="acc0")
                        s = p1c.tile([P, G_H], F32, tag="acc1")
                        nc.vector.tensor_tensor(a[:], terms[0][0][:], terms[1][0][:],
                                                ALU.add if terms[1][1] > 0 else ALU.subtract)
                        nc.vector.tensor_tensor(s[:], a[:], terms[2][0][:],
                                                ALU.add if terms[2][1] > 0 else ALU.subtract)
                        nc.vector.scalar_tensor_tensor(
                            wmg[:, :, k], s[:], float(thr), wg[:, :, k],
                            op, ALU.mult)

                    # keep_0: ge01+ge02+ge03 >= 2  (thr 1.5, is_ge)
                    keep_apply(0, [(ge01, 1), (ge02, 1), (ge03, 1)], 1.5, ALU.is_ge)
                    # keep_1: ge12+ge13-ge01 >= 1  (thr 0.5, is_ge)
                    keep_apply(1, [(ge12, 1), (ge13, 1), (ge01, -1)], 0.5, ALU.is_ge)
                    # keep_2: ge23-ge02-ge12 >= 0  (thr -0.5, is_ge)
                    keep_apply(2, [(ge23, 1), (ge02, -1), (ge12, -1)], -0.5, ALU.is_ge)
                    # keep_3: ge03+ge13+ge23 <= 1  (thr 1.5, is_le)
                    keep_apply(3, [(ge03, 1), (ge13, 1), (ge23, 1)], 1.5, ALU.is_le)

                    # transpose masked weight into wmt[p, k, o-block]
                    n_kk = D_HALF // P
                    for kk in range(n_kk):
                        kabs = (d0 // P) + kk
                        if MM_IS_F32:
                            # no 4-byte xbar transpose; handled via PE below
                            raise NotImplementedError(
                                "f32/f32r weight transpose path not built")
                        nc.sync.dma_start_transpose(
                            out=wmt[:, kabs, o0:o0 + P],
                            in_=wmb[:, kk * P:(kk + 1) * P])

        # --- phase 2: out[s_blk, :] = x[s_blk, :] @ Wm^T + bias ---
        with (
            tc.tile_pool(name="xstage", bufs=3) as xstage,
            tc.tile_pool(name="xb", bufs=2) as xbp,
            tc.tile_pool(name="osb", bufs=4) as osb,
            tc.tile_pool(name="ps", bufs=8, space="PSUM") as ps,
        ):
            # xt viewed so partition p picks d = 128k + p
            xt_r = xt.rearrange("(kb kk p) s -> kb p kk s", kk=KG, p=P)
            for blk in range(N_BLK):
                s0 = blk * S_BLK
                xb = xbp.tile([P, K_TILES, S_BLK], MM_DT, tag="xb")
                for kg in range(K_TILES // KG):
                    xs = xstage.tile([P, KG, S_BLK], F32, tag="xs")
                    nc.sync.dma_start(out=xs[:], in_=xt_r[kg, :, :, s0:s0 + S_BLK])
                    nc.vector.tensor_copy(xb[:, kg * KG:(kg + 1) * KG, :], xs[:])
                for st in range(S_BLK // P):
                    psum = ps.tile([P, O_SH], F32, tag="ps")
                    for k in range(K_TILES):
                        nc.tensor.matmul(
                            psum[:],
                            xb[:, k, st * P:(st + 1) * P],
                            wmt[:, k, :],
                            start=(k == 0), stop=False)
                    nc.tensor.matmul(psum[:], ones1[:], biasT[:],
                                     start=False, stop=True)
                    o_sb = osb.tile([P, O_SH], F32, tag="osb")
                    nc.scalar.copy(o_sb[:], psum[:])
                    nc.sync.dma_start(
                        out=out[s0 + st * P: s0 + (st + 1) * P, :],
                        in_=o_sb[:])
    nc.compile()
    return nc


def _get_nc():
    if "nc" not in _CACHED:
        _CACHED["nc"] = _build_nc()
    return _CACHED["nc"]


def kernel(x, weight, bias, scores, noise_u):
    global LAST_EXEC_NS
    x = np.ascontiguousarray(np.asarray(x, dtype=np.float32))
    weight = np.ascontiguousarray(np.asarray(weight, dtype=np.float32))
    bias = np.ascontiguousarray(np.asarray(bias, dtype=np.float32))
    scores = np.asarray(scores, dtype=np.float32).reshape(D_OUT, D_IN)
    noise_u = np.asarray(noise_u, dtype=np.float32).reshape(D_OUT, D_IN)

    # pure relayout: contraction dim onto rows (so it maps to partitions)
    xT = np.ascontiguousarray(x.reshape(S_TOT, D_IN).T)

    in_maps = []
    for j in range(N_CORES):
        o0 = j * O_SH
        in_maps.append({
            "xt": xT,
            "wsh": np.ascontiguousarray(weight[o0:o0 + O_SH]),
            "ssh": np.ascontiguousarray(scores[o0:o0 + O_SH]),
            "nsh": np.ascontiguousarray(noise_u[o0:o0 + O_SH]),
            "bsh": np.ascontiguousarray(bias[o0:o0 + O_SH]).reshape(1, O_SH),
        })

    nc = _get_nc()
    if os.environ.get("BASS_KERNEL_TIMED", "0") == "1":
        results, exec_ns = _run_timed(nc, in_maps)
        LAST_EXEC_NS = exec_ns
    else:
        res = run_bass_kernel_spmd(nc, in_maps, list(range(N_CORES)), trace=False)
        LAST_EXEC_NS = res.exec_time_ns
        results = res.results
    out = np.concatenate(
        [np.asarray(results[j]["out"]) for j in range(N_CORES)], axis=1)
    return out.reshape(B, S, D_OUT).astype(np.float32)


def _run_timed(nc, in_maps, n_iters=8):
    """Mimic bass2jax.run_bass_via_pjrt multi-core path, but keep inputs
    device-resident and time pipelined repeat executions."""
    import time
    import jax
    from jax.sharding import Mesh, PartitionSpec, NamedSharding
    from jax.experimental.shard_map import shard_map
    from concourse import bass2jax, mybir as _mb

    bass2jax.install_neuronx_cc_hook()
    n_cores = len(in_maps)
    partition_name = (nc.partition_id_tensor.name
                      if nc.partition_id_tensor else None)
    in_names, out_names, out_avals = [], [], []
    for alloc in nc.m.functions[0].allocations:
        if not isinstance(alloc, _mb.MemoryLocationSet):
            continue
        name = alloc.memorylocations[0].name
        if alloc.kind == "ExternalInput":
            if name != partition_name:
                in_names.append(name)
        elif alloc.kind == "ExternalOutput":
            out_names.append(name)
            out_avals.append(jax.core.ShapedArray(
                tuple(alloc.tensor_shape), _mb.dt.np(alloc.dtype)))
    n_params = len(in_names)
    all_names = in_names + out_names + ([partition_name] if partition_name else [])

    def _body(*args):
        operands = list(args)
        if partition_name is not None:
            operands.append(bass2jax.partition_id_tensor())
        return tuple(bass2jax._bass_exec_p.bind(
            *operands, out_avals=tuple(out_avals), in_names=tuple(all_names),
            out_names=tuple(out_names), lowering_input_output_aliases=(),
            sim_require_finite=True, sim_require_nnan=True, nc=nc))

    devices = jax.devices()[:n_cores]
    mesh = Mesh(np.array(devices), ("core",))
    spec = PartitionSpec("core")
    n_outs = len(out_names)
    fn = jax.jit(shard_map(_body, mesh=mesh,
                           in_specs=(spec,) * (n_params + n_outs),
                           out_specs=(spec,) * n_outs, check_rep=False),
                 keep_unused=True)
    sh = NamedSharding(mesh, spec)
    ins_dev = [jax.device_put(
        np.concatenate([np.asarray(m[nm]) for m in in_maps], axis=0), sh)
        for nm in in_names]
    zeros_dev = [jax.device_put(
        np.zeros((n_cores * a.shape[0], *a.shape[1:]), a.dtype), sh)
        for a in out_avals]
    outs = fn(*ins_dev, *zeros_dev)     # compile + warm
    jax.block_until_ready(outs)
    t0 = time.perf_counter()
    for _ in range(n_iters):
        last = fn(*ins_dev, *zeros_dev)  # pipelined async dispatch
    jax.block_until_ready(last)
    dt_ns = (time.perf_counter() - t0) / n_iters * 1e9
    results = [
        {nm: np.asarray(last[i]).reshape(n_cores, *out_avals[i].shape)[c]
         for i, nm in enumerate(out_names)}
        for c in range(n_cores)]
    return results, int(dt_ns)


# revision 12
# speedup vs baseline: 3.7958x; 3.7958x over previous
"""Trainium2 Bass kernel for HDSLinear (gumbel top-2-of-4 masked linear).

Strategy (column-parallel, per sharding hint):
  - Shard weight/scores/noise_u/bias along out_features across 8 cores
    (512 rows each); replicate x (uploaded transposed: [d_in, s] layout,
    a pure host-side relayout so the contraction dim lands on SBUF
    partitions without any device-side transpose).
  - Each core computes its mask shard from scores+gumbel noise on device
    (ACT: 2x Ln; DVE: pairwise-compare rank select), applies it to the
    weight shard, transposes the masked weight on-chip (xbar DMA
    transpose, bf16), then runs x @ Wm^T as a PE matmul accumulating
    over d_in, + bias via a K=1 matmul, and streams out [16384, 512].
  - Host concatenates the 8 output shards along out_features.

Matmul dtype is bf16 by default (BASS_MM_MODE=bf16|f32r|fp32).
"""

import os
import sys
import numpy as np
from contextlib import ExitStack

for _p in ("/opt/trn_rl_repo", "/root/.axon_site/_ro/trn_rl_repo"):
    if os.path.isdir(_p) and _p not in sys.path:
        sys.path.insert(0, _p)

import concourse.bass as bass
import concourse.bacc as bacc
import concourse.mybir as mybir
from concourse import tile
from concourse.bass_utils import run_bass_kernel_spmd

F32 = mybir.dt.float32
BF16 = mybir.dt.bfloat16
AF = mybir.ActivationFunctionType
ALU = mybir.AluOpType

B, S, D_IN, D_OUT = 8, 2048, 4096, 4096
N_CORES = 8
S_TOT = B * S                      # 16384
O_SH = D_OUT // N_CORES            # 512 out-features per core
P = 128
EPS = 1e-10

MM_MODE = os.environ.get("BASS_MM_MODE", "bf16")
MM_DT = {"bf16": BF16, "f32r": mybir.dt.float32r, "fp32": F32}[MM_MODE]
MM_IS_F32 = MM_MODE in ("f32r", "fp32")

K_TILES = D_IN // P                # 32 contraction tiles
S_BLK = 256                        # s-columns per phase-2 block
N_BLK = S_TOT // S_BLK             # 32 blocks
KG = 8                             # k-tiles per x staging DMA
O_TILES = O_SH // P                # 4 o-tiles of 128 rows in phase 1
D_HALF = 1024                      # phase-1 d-chunk width

LAST_EXEC_NS = None
_CACHED = {}


def _build_nc():
    nc = bacc.Bacc(None, target_bir_lowering=False)
    xt = nc.declare_dram_parameter("xt", [D_IN, S_TOT], F32, isOutput=False)
    wsh = nc.declare_dram_parameter("wsh", [O_SH, D_IN], F32, isOutput=False)
    ssh = nc.declare_dram_parameter("ssh", [O_SH, D_IN], F32, isOutput=False)
    nsh = nc.declare_dram_parameter("nsh", [O_SH, D_IN], F32, isOutput=False)
    bsh = nc.declare_dram_parameter("bsh", [1, O_SH], F32, isOutput=False)
    out = nc.declare_dram_parameter("out", [S_TOT, O_SH], F32, isOutput=True)

    with tile.TileContext(nc) as tc:
      with tc.tile_pool(name="const", bufs=1) as const:
        # --- persistent tiles ---
        # Masked weight, transposed: wmt[p, k, o] = Wm[o, 128k+p]
        wmt = const.tile([P, K_TILES, O_SH], MM_DT, tag="wmt")
        ones1 = const.tile([1, P], MM_DT, tag="ones1")
        nc.any.memset(ones1[:], 1.0)
        biasT = const.tile([1, O_SH], MM_DT, tag="biasT")
        bias_f32 = const.tile([1, O_SH], F32, tag="bias_f32")
        nc.sync.dma_start(out=bias_f32[:], in_=bsh[:, :])
        nc.vector.tensor_copy(biasT[:], bias_f32[:])
        epsb = const.tile([P, 1], F32, tag="epsb")
        nc.any.memset(epsb[:], EPS)

        # --- phase 1: mask generation + masked weight (transposed) ---
        with (
            tc.tile_pool(name="p1io", bufs=2) as p1io,
            tc.tile_pool(name="p1t", bufs=2) as p1t,
            tc.tile_pool(name="p1c", bufs=2) as p1c,
            tc.tile_pool(name="xstage", bufs=3) as xstage,
            tc.tile_pool(name="xb", bufs=2) as xbp,
            tc.tile_pool(name="osb", bufs=4) as osb,
            tc.tile_pool(name="ps", bufs=8, space="PSUM") as ps,
        ):
            n_half = D_IN // D_HALF
            G_H = D_HALF // 4   # groups per half-chunk
            for ot in range(O_TILES):
                o0 = ot * P
                for h in range(n_half):
                    d0 = h * D_HALF
                    sc = p1io.tile([P, D_HALF], F32, tag="sc")
                    nu = p1io.tile([P, D_HALF], F32, tag="nu")
                    w = p1io.tile([P, D_HALF], F32, tag="w")
                    nc.sync.dma_start(out=sc[:], in_=ssh[o0:o0 + P, d0:d0 + D_HALF])
                    nc.sync.dma_start(out=nu[:], in_=nsh[o0:o0 + P, d0:d0 + D_HALF])
                    nc.sync.dma_start(out=w[:], in_=wsh[o0:o0 + P, d0:d0 + D_HALF])

                    t1 = p1t.tile([P, D_HALF], F32, tag="t1")
                    t2 = p1t.tile([P, D_HALF], F32, tag="t2")
                    y = p1t.tile([P, D_HALF], F32, tag="y")
                    wmb = p1t.tile([P, D_HALF], MM_DT, tag="wmb")
                    # gumbel chain, mirroring jax fp32 op order:
                    # t1 = ln(u + eps); t2 = ln(-t1 + eps); y = scores - t2
                    nc.scalar.activation(t1[:], nu[:], AF.Ln, bias=epsb[:])
                    nc.scalar.activation(t2[:], t1[:], AF.Ln, bias=epsb[:], scale=-1.0)
                    nc.vector.tensor_sub(y[:], sc[:], t2[:])

                    yg = y.rearrange("p (g m) -> p g m", m=4)
                    wg = w.rearrange("p (g m) -> p g m", m=4)
                    wmg = wmb.rearrange("p (g m) -> p g m", m=4)
                    yk = [yg[:, :, k] for k in range(4)]

                    def cmp(a, b):
                        t = p1c.tile([P, G_H], F32, tag=f"ge{a}{b}")
                        nc.vector.tensor_tensor(t[:], yk[a][:], yk[b][:], ALU.is_ge)
                        return t

                    ge01, ge02, ge03 = cmp(0, 1), cmp(0, 2), cmp(0, 3)
                    ge12, ge13, ge23 = cmp(1, 2), cmp(1, 3), cmp(2, 3)

                    def keep_apply(k, terms, thr, op):
                        # sum(terms) (with signs) `op` thr -> *w_k -> wm_k
                        a = p1c.tile([P, G_H], F32, tag="acc0")
                        s = p1c.tile([P, G_H], F32, tag="acc1")
                        nc.vector.tensor_tensor(a[:], terms[0][0][:], terms[1][0][:],
                                                ALU.add if terms[1][1] > 0 else ALU.subtract)
                        nc.vector.tensor_tensor(s[:], a[:], terms[2][0][:],
                                                ALU.add if terms[2][1] > 0 else ALU.subtract)
                        nc.vector.scalar_tensor_tensor(
                            wmg[:, :, k], s[:], float(thr), wg[:, :, k],
                            op, ALU.mult)

                    # keep_0: ge01+ge02+ge03 >= 2  (thr 1.5, is_ge)
                    keep_apply(0, [(ge01, 1), (ge02, 1), (ge03, 1)], 1.5, ALU.is_ge)
                    # keep_1: ge12+ge13-ge01 >= 1  (thr 0.5, is_ge)
                    keep_apply(1, [(ge12, 1), (ge13, 1), (ge01, -1)], 0.5, ALU.is_ge)
                    # keep_2: ge23-ge02-ge12 >= 0  (thr -0.5, is_ge)
                    keep_apply(2, [(ge23, 1), (ge02, -1), (ge12, -1)], -0.5, ALU.is_ge)
                    # keep_3: ge03+ge13+ge23 <= 1  (thr 1.5, is_le)
                    keep_apply(3, [(ge03, 1), (ge13, 1), (ge23, 1)], 1.5, ALU.is_le)

                    # transpose masked weight into wmt[p, k, o-block]
                    n_kk = D_HALF // P
                    for kk in range(n_kk):
                        kabs = (d0 // P) + kk
                        if MM_IS_F32:
                            # no 4-byte xbar transpose; handled via PE below
                            raise NotImplementedError(
                                "f32/f32r weight transpose path not built")
                        nc.sync.dma_start_transpose(
                            out=wmt[:, kabs, o0:o0 + P],
                            in_=wmb[:, kk * P:(kk + 1) * P])

            # --- phase 2: out[s_blk, :] = x[s_blk, :] @ Wm^T + bias ---
            # (same pool scope as phase 1 so the scheduler overlaps x
            #  prefetch/casts with mask generation; casts on GPSIMD keep
            #  the DVE free for the mask compares)
            # xt viewed so partition p picks d = 128k + p
            xt_r = xt.rearrange("(kb kk p) s -> kb p kk s", kk=KG, p=P)
            for blk in range(N_BLK):
                s0 = blk * S_BLK
                xb = xbp.tile([P, K_TILES, S_BLK], MM_DT, tag="xb")
                for kg in range(K_TILES // KG):
                    xs = xstage.tile([P, KG, S_BLK], F32, tag="xs")
                    nc.sync.dma_start(out=xs[:], in_=xt_r[kg, :, :, s0:s0 + S_BLK])
                    nc.gpsimd.tensor_copy(xb[:, kg * KG:(kg + 1) * KG, :], xs[:])
                for st in range(S_BLK // P):
                    psum = ps.tile([P, O_SH], F32, tag="ps")
                    for k in range(K_TILES):
                        nc.tensor.matmul(
                            psum[:],
                            xb[:, k, st * P:(st + 1) * P],
                            wmt[:, k, :],
                            start=(k == 0), stop=False)
                    nc.tensor.matmul(psum[:], ones1[:], biasT[:],
                                     start=False, stop=True)
                    o_sb = osb.tile([P, O_SH], F32, tag="osb")
                    nc.scalar.copy(o_sb[:], psum[:])
                    nc.sync.dma_start(
                        out=out[s0 + st * P: s0 + (st + 1) * P, :],
                        in_=o_sb[:])
    nc.compile()
    return nc


def _get_nc():
    if "nc" not in _CACHED:
        _CACHED["nc"] = _build_nc()
    return _CACHED["nc"]


def kernel(x, weight, bias, scores, noise_u):
    global LAST_EXEC_NS
    x = np.ascontiguousarray(np.asarray(x, dtype=np.float32))
    weight = np.ascontiguousarray(np.asarray(weight, dtype=np.float32))
    bias = np.ascontiguousarray(np.asarray(bias, dtype=np.float32))
    scores = np.asarray(scores, dtype=np.float32).reshape(D_OUT, D_IN)
    noise_u = np.asarray(noise_u, dtype=np.float32).reshape(D_OUT, D_IN)

    # pure relayout: contraction dim onto rows (so it maps to partitions)
    xT = np.ascontiguousarray(x.reshape(S_TOT, D_IN).T)

    in_maps = []
    for j in range(N_CORES):
        o0 = j * O_SH
        in_maps.append({
            "xt": xT,
            "wsh": np.ascontiguousarray(weight[o0:o0 + O_SH]),
            "ssh": np.ascontiguousarray(scores[o0:o0 + O_SH]),
            "nsh": np.ascontiguousarray(noise_u[o0:o0 + O_SH]),
            "bsh": np.ascontiguousarray(bias[o0:o0 + O_SH]).reshape(1, O_SH),
        })

    nc = _get_nc()
    if os.environ.get("BASS_KERNEL_TIMED", "0") == "1":
        results, exec_ns = _run_timed(nc, in_maps)
        LAST_EXEC_NS = exec_ns
    else:
        res = run_bass_kernel_spmd(nc, in_maps, list(range(N_CORES)), trace=False)
        LAST_EXEC_NS = res.exec_time_ns
        results = res.results
    out = np.concatenate(
        [np.asarray(results[j]["out"]) for j in range(N_CORES)], axis=1)
    return out.reshape(B, S, D_OUT).astype(np.float32)


def _run_timed(nc, in_maps, n_iters=64):
    """Mimic bass2jax.run_bass_via_pjrt multi-core path, but keep inputs
    device-resident and time pipelined repeat executions."""
    import time
    import jax
    from jax.sharding import Mesh, PartitionSpec, NamedSharding
    from jax.experimental.shard_map import shard_map
    from concourse import bass2jax, mybir as _mb

    bass2jax.install_neuronx_cc_hook()
    n_cores = len(in_maps)
    partition_name = (nc.partition_id_tensor.name
                      if nc.partition_id_tensor else None)
    in_names, out_names, out_avals = [], [], []
    for alloc in nc.m.functions[0].allocations:
        if not isinstance(alloc, _mb.MemoryLocationSet):
            continue
        name = alloc.memorylocations[0].name
        if alloc.kind == "ExternalInput":
            if name != partition_name:
                in_names.append(name)
        elif alloc.kind == "ExternalOutput":
            out_names.append(name)
            out_avals.append(jax.core.ShapedArray(
                tuple(alloc.tensor_shape), _mb.dt.np(alloc.dtype)))
    n_params = len(in_names)
    all_names = in_names + out_names + ([partition_name] if partition_name else [])

    def _body(*args):
        operands = list(args)
        if partition_name is not None:
            operands.append(bass2jax.partition_id_tensor())
        return tuple(bass2jax._bass_exec_p.bind(
            *operands, out_avals=tuple(out_avals), in_names=tuple(all_names),
            out_names=tuple(out_names), lowering_input_output_aliases=(),
            sim_require_finite=True, sim_require_nnan=True, nc=nc))

    devices = jax.devices()[:n_cores]
    mesh = Mesh(np.array(devices), ("core",))
    spec = PartitionSpec("core")
    n_outs = len(out_names)
    fn = jax.jit(shard_map(_body, mesh=mesh,
                           in_specs=(spec,) * (n_params + n_outs),
                           out_specs=(spec,) * n_outs, check_rep=False),
                 keep_unused=True)
    sh = NamedSharding(mesh, spec)
    ins_dev = [jax.device_put(
        np.concatenate([np.asarray(m[nm]) for m in in_maps], axis=0), sh)
        for nm in in_names]
    zeros_dev = [jax.device_put(
        np.zeros((n_cores * a.shape[0], *a.shape[1:]), a.dtype), sh)
        for a in out_avals]
    outs = fn(*ins_dev, *zeros_dev)     # compile + warm
    jax.block_until_ready(outs)
    t0 = time.perf_counter()
    for _ in range(n_iters):
        last = fn(*ins_dev, *zeros_dev)  # pipelined async dispatch
    jax.block_until_ready(last)
    dt_ns = (time.perf_counter() - t0) / n_iters * 1e9
    results = [
        {nm: np.asarray(last[i]).reshape(n_cores, *out_avals[i].shape)[c]
         for i, nm in enumerate(out_names)}
        for c in range(n_cores)]
    return results, int(dt_ns)
